# revision 5
# baseline (speedup 1.0000x reference)
"""Conv4d (valid, stride 1) on 8 Trainium2 NeuronCores via Bass.

Problem (hardcoded):
  x      [2, 16, 16, 16, 32, 32]  (B, C_in, T, D, H, W)  fp32
  weight [32, 16, 3, 3, 3, 3]     (C_out, C_in, kT, kD, kH, kW)
  bias   [32]
  y      [2, 32, 14, 14, 30, 30]  (B, C_out, oT, oD, oH, oW)

Strategy: banded-matmul formulation on the PE array, fp32r (full-rate).
Each output tile is a 2x2 block in (oT, oH) x all (oD, oW):
  matmul out[m, n], m = (dt2, dh2, c_out32) = 128, n = (od14, ow30) = 420
  K = 128 = (ci8, ti4, hi4): half of C_in x a 4-wide t-window x 4-wide h-window.
  The kt / kh taps live in a banded stationary weight (lhsT[k, m] = w[co, ci,
  ti-dt, kd, hi-dh, kw], zero off-band); kd, kw, c_in-half are 18
  PSUM-accumulation rounds whose input shifts are free-dim AP offsets.
  -> 72 useful MACs per PE column = 56.25% utilization.

Tiles: (b2, bt7, bh15) = 210, padded to 216 -> 27 per core (pads are
duplicates of real tiles; their outputs are identical so scatter is safe).

Host pre-packs per-core slabs so every DMA is fully contiguous per
partition; device does DMA-in -> 18 matmuls -> ACT bias-copy -> DMA-out,
software-pipelined with explicit semaphores (raw Bass, no Tile).
"""

import os

import numpy as np

B, C, TT, DD, HH, WW = 2, 16, 16, 16, 32, 32
CO = 32
K = 3
OT, OD, OH, OW = 14, 14, 30, 30
NBT, NBH = OT // 2, OH // 2  # 7, 15 output 2x2 blocks in (t, h)
N_CORES = 8
NTILES = B * NBT * NBH  # 210
TILES_PER_CORE = (NTILES + N_CORES - 1) // N_CORES  # 27
NPAD = TILES_PER_CORE * N_CORES  # 216
N_ROUNDS = K * K * 2  # (kd, kw, half) = 18

NSLAB = 3  # slab double/triple buffering
NPSUM = 4
NY = 3

LAST_EXEC_TIME_NS = None
_PROGRAM = None


def _tile_list():
    tiles = [
        (b, bt, bh) for b in range(B) for bt in range(NBT) for bh in range(NBH)
    ]
    while len(tiles) < NPAD:
        tiles.append(tiles[0])
    return [tiles[c::N_CORES] for c in range(N_CORES)]


def _build_program():
    import concourse.bass as bass
    import concourse.mybir as mybir

    f32 = mybir.dt.float32
    f32r = mybir.dt.float32r

    nc = bass.Bass("TRN2", target_bir_lowering=False, debug=False)
    xslab_d = nc.dram_tensor(
        "xslab", [TILES_PER_CORE, 2, 128, DD, WW], f32r, kind="ExternalInput"
    )
    wband_d = nc.dram_tensor("wband", [N_ROUNDS, 128, 128], f32r, kind="ExternalInput")
    bias_d = nc.dram_tensor("bias128", [128, 1], f32, kind="ExternalInput")
    yout_d = nc.dram_tensor(
        "yout", [TILES_PER_CORE, 128, OD, OW], f32, kind="ExternalOutput"
    )

    from contextlib import ExitStack

    with ExitStack() as ctx:
        slabs = [
            ctx.enter_context(nc.sbuf_tensor(f"slab{j}", [128, 2, DD, WW], f32r))
            for j in range(NSLAB)
        ]
        wband_sb = ctx.enter_context(
            nc.sbuf_tensor("wband_sb", [128, N_ROUNDS, 128], f32r)
        )
        bias_sb = ctx.enter_context(nc.sbuf_tensor("bias_sb", [128, 1], f32))
        ys = [
            ctx.enter_context(nc.sbuf_tensor(f"ysb{j}", [128, OD, OW], f32))
            for j in range(NY)
        ]
        psums = [
            ctx.enter_context(nc.psum_tensor(f"ps{j}", [128, OD, OW], f32))
            for j in range(NPSUM)
        ]
        # Per-buffer DMA-completion semaphores: HWDGE queues complete out of
        # order, so a shared counting semaphore waited at a prefix value is
        # unsafe. Each buffer's semaphore only ever has one DMA in flight
        # (reuse is guarded by pe/act sems), so exact totals are race-free.
        slab_sems = [
            ctx.enter_context(nc.semaphore(f"slab_dma{j}")) for j in range(NSLAB)
        ]
        yout_sems = [
            ctx.enter_context(nc.semaphore(f"yout_dma{j}")) for j in range(NY)
        ]
        wband_sem = ctx.enter_context(nc.semaphore("wband_dma"))
        bias_sem = ctx.enter_context(nc.semaphore("bias_dma"))
        pe_sem = ctx.enter_context(nc.semaphore("pe_done"))
        act_sem = ctx.enter_context(nc.semaphore("act_done"))
        block = ctx.enter_context(nc.Block())

        @block.sync
        def _(sync):
            for r in range(N_ROUNDS):
                sync.dma_start(
                    out=wband_sb.ap()[:, r], in_=wband_d.ap()[r]
                ).then_inc(wband_sem, 16)
            sync.dma_start(out=bias_sb.ap()[:], in_=bias_d.ap()[:]).then_inc(
                bias_sem, 16
            )
            for i in range(TILES_PER_CORE):
                if i >= NSLAB:
                    # slab buffer reused: PE must be done reading tile i-NSLAB
                    sync.wait_ge(pe_sem, i - NSLAB + 1)
                sync.dma_start(
                    out=slabs[i % NSLAB].ap()[:],
                    in_=xslab_d.ap()[i].rearrange("h p d w -> p h d w"),
                ).then_inc(slab_sems[i % NSLAB], 16)

        @block.tensor
        def _(tensor):
            tensor.wait_ge(wband_sem, 16 * N_ROUNDS)  # all wband loaded
            for i in range(TILES_PER_CORE):
                tensor.wait_ge(slab_sems[i % NSLAB], 16 * (i // NSLAB + 1))
                if i >= NPSUM:
                    tensor.wait_ge(act_sem, i - NPSUM + 1)  # psum bank free
                ps = psums[i % NPSUM].ap()[:]
                sl = slabs[i % NSLAB].ap()
                mm = None
                r = 0
                for kd in range(K):
                    for kw in range(K):
                        for half in range(2):
                            mm = tensor.matmul(
                                ps,
                                wband_sb.ap()[:, r],
                                sl[:, half, kd : kd + OD, kw : kw + OW],
                                start=(r == 0),
                                stop=(r == N_ROUNDS - 1),
                            )
                            r += 1
                mm.then_inc(pe_sem, 1)

        @block.scalar
        def _(scalar):
            scalar.wait_ge(bias_sem, 16)  # bias loaded
            for i in range(TILES_PER_CORE):
                scalar.wait_ge(pe_sem, i + 1)
                if i >= NY:
                    scalar.wait_ge(yout_sems[i % NY], 16 * (i // NY))  # ysb free
                import concourse.mybir as mybir_

                scalar.activation(
                    ys[i % NY].ap()[:],
                    psums[i % NPSUM].ap()[:],
                    mybir_.ActivationFunctionType.Identity,
                    bias=bias_sb.ap()[:],
                ).then_inc(act_sem, 1)

        @block.gpsimd
        def _(gpsimd):
            for i in range(TILES_PER_CORE):
                gpsimd.wait_ge(act_sem, i + 1)
                gpsimd.dma_start(
                    out=yout_d.ap()[i], in_=ys[i % NY].ap()[:]
                ).then_inc(yout_sems[i % NY], 16)

    return nc


def _get_program():
    global _PROGRAM
    if _PROGRAM is None:
        _PROGRAM = _build_program()
    return _PROGRAM


def _prep_inputs(x, weight, bias):
    per_core_tiles = _tile_list()
    xslabs = []
    for core_tiles in per_core_tiles:
        xs = np.empty((TILES_PER_CORE, 2, 128, DD, WW), dtype=np.float32)
        for idx, (b, bt, bh) in enumerate(core_tiles):
            blk = x[b, :, 2 * bt : 2 * bt + 4, :, 2 * bh : 2 * bh + 4, :]
            # (c, t, d, h, w) -> (c, t, h, d, w)
            blk = blk.transpose(0, 1, 3, 2, 4)
            xs[idx] = blk.reshape(2, 8, 4, 4, DD, WW).reshape(2, 128, DD, WW)
        xslabs.append(xs)

    wband = np.zeros((N_ROUNDS, 8, 4, 4, 2, 2, CO), dtype=np.float32)
    r = 0
    for kd in range(K):
        for kw in range(K):
            for half in range(2):
                for dt in range(2):
                    for ti in range(4):
                        kt = ti - dt
                        if not (0 <= kt < K):
                            continue
                        for dh in range(2):
                            for hi in range(4):
                                kh = hi - dh
                                if not (0 <= kh < K):
                                    continue
                                # lhsT[(ci,ti,hi),(dt,dh,co)] = w[co, half*8+ci, kt, kd, kh, kw]
                                wband[r, :, ti, hi, dt, dh, :] = weight[
                                    :, half * 8 : half * 8 + 8, kt, kd, kh, kw
                                ].T
                r += 1
    wband = wband.reshape(N_ROUNDS, 128, 128)

    bias128 = np.tile(bias.astype(np.float32), 4).reshape(128, 1)
    return xslabs, wband, bias128, per_core_tiles


def _scatter_outputs(youts, per_core_tiles):
    y = np.empty((B, CO, OT, OD, OH, OW), dtype=np.float32)
    for c in range(N_CORES):
        yc = youts[c].reshape(TILES_PER_CORE, 2, 2, CO, OD, OW)
        for idx, (b, bt, bh) in enumerate(per_core_tiles[c]):
            for dt in range(2):
                for dh in range(2):
                    y[b, :, 2 * bt + dt, :, 2 * bh + dh, :] = yc[idx, dt, dh]
    return y


def kernel(x, weight, bias):
    global LAST_EXEC_TIME_NS
    x = np.asarray(x, dtype=np.float32)
    weight = np.asarray(weight, dtype=np.float32)
    bias = np.asarray(bias, dtype=np.float32)

    xslabs, wband, bias128, per_core_tiles = _prep_inputs(x, weight, bias)
    nc = _get_program()

    from concourse.bass_utils import run_bass_kernel_spmd

    in_maps = [
        {"xslab": xslabs[c], "wband": wband, "bias128": bias128}
        for c in range(N_CORES)
    ]
    trace = os.environ.get("CONV4D_TRACE") == "1"
    res = run_bass_kernel_spmd(nc, in_maps, list(range(N_CORES)), trace=trace)
    LAST_EXEC_TIME_NS = res.exec_time_ns

    youts = [res.results[c]["yout"] for c in range(N_CORES)]
    return _scatter_outputs(youts, per_core_tiles)


# revision 12
# speedup vs baseline: 1.0192x; 1.0192x over previous
"""Conv4d (valid, stride 1) on 8 Trainium2 NeuronCores via Bass.

Problem (hardcoded):
  x      [2, 16, 16, 16, 32, 32]  (B, C_in, T, D, H, W)  fp32
  weight [32, 16, 3, 3, 3, 3]     (C_out, C_in, kT, kD, kH, kW)
  bias   [32]
  y      [2, 32, 14, 14, 30, 30]  (B, C_out, oT, oD, oH, oW)

Strategy: banded-matmul formulation on the PE array, fp32r (full-rate).
Each output tile is a 2x2 block in (oT, oH) x all of (oD, oW):
  matmul out[m, n], m = (dt2, dh2, c_out32) = 128, n = (od14, ow30) = 420
  K = 128 = (ci8, ti4, hi4): half of C_in x 4-wide t-window x 4-wide h-window.
  kt / kh taps live in a banded stationary weight (lhsT[k, m] = w[co, ci,
  ti-dt, kd, hi-dh, kw], zero off-band); kd, kw, c_in-half are 18
  PSUM-accumulation rounds whose input shifts are free-dim AP offsets.
  -> 72 useful MACs per PE column = 56.25% PE utilization.

Tiles: (b2, bt7, bh15) = 210, padded to 216 -> 27 per core (pads duplicate
real tiles; identical outputs make the scatter order-safe).

Host pre-packs per-core slabs so every DMA moves >=2KB contiguous per
partition; device pipeline: DMA-in -> 18 matmuls -> ACT bias-copy ->
DMA-out, with explicit semaphores (raw Bass, no Tile). Tiles are processed
in groups of 4 across 8 PSUM banks with rounds outer so consecutive
matmuls share the stationary operand; warmup matmuls keep the PE HAM
clock-gate open during the initial DMA ramp.
"""

import os

import numpy as np

B, C, TT, DD, HH, WW = 2, 16, 16, 16, 32, 32
CO = 32
K = 3
OT, OD, OH, OW = 14, 14, 30, 30
NBT, NBH = OT // 2, OH // 2  # 7, 15 output 2x2 blocks in (t, h)
N_CORES = 8
NTILES = B * NBT * NBH  # 210
TILES_PER_CORE = (NTILES + N_CORES - 1) // N_CORES  # 27
NPAD = TILES_PER_CORE * N_CORES  # 216
N_ROUNDS = K * K * 2  # (kd, kw, half) = 18

GROUP = 4  # tiles accumulated concurrently (one PSUM bank each)
NPSUM = 8
NSLAB = 2 * GROUP  # active group + prefetch group
NY = 4
N_WARMUP = 8  # PE warmup matmuls (N=240 fp32r) during initial DMA ramp

LAST_EXEC_TIME_NS = None
_PROGRAM = None


def _tile_list():
    tiles = [
        (b, bt, bh) for b in range(B) for bt in range(NBT) for bh in range(NBH)
    ]
    while len(tiles) < NPAD:
        tiles.append(tiles[0])
    return [tiles[c::N_CORES] for c in range(N_CORES)]


def _build_program():
    import concourse.bass as bass
    import concourse.mybir as mybir

    f32 = mybir.dt.float32
    f32r = mybir.dt.float32r

    nc = bass.Bass("TRN2", target_bir_lowering=False, debug=False)
    xslab_d = nc.dram_tensor(
        "xslab", [TILES_PER_CORE, 2, 128, DD, WW], f32r, kind="ExternalInput"
    )
    # partition-major: one DMA, 128 x 9216B contiguous lines
    wband_d = nc.dram_tensor(
        "wband", [128, N_ROUNDS, 128], f32r, kind="ExternalInput"
    )
    bias_d = nc.dram_tensor("bias128", [128, 1], f32, kind="ExternalInput")
    yout_d = nc.dram_tensor(
        "yout", [TILES_PER_CORE, 128, OD, OW], f32, kind="ExternalOutput"
    )

    n_groups = (TILES_PER_CORE + GROUP - 1) // GROUP

    from contextlib import ExitStack

    with ExitStack() as ctx:
        slabs = [
            ctx.enter_context(nc.sbuf_tensor(f"slab{j}", [128, 2, DD, WW], f32r))
            for j in range(NSLAB)
        ]
        wband_sb = ctx.enter_context(
            nc.sbuf_tensor("wband_sb", [128, N_ROUNDS, 128], f32r)
        )
        # warmup matmul operands (values irrelevant, zeroed for the sim)
        wmbuf = ctx.enter_context(nc.sbuf_tensor("wmbuf", [128, 368], f32))
        bias_sb = ctx.enter_context(nc.sbuf_tensor("bias_sb", [128, 1], f32))
        ys = [
            ctx.enter_context(nc.sbuf_tensor(f"ysb{j}", [128, OD, OW], f32))
            for j in range(NY)
        ]
        psums = [
            ctx.enter_context(nc.psum_tensor(f"ps{j}", [128, OD, OW], f32))
            for j in range(NPSUM)
        ]
        # Per-buffer DMA-completion semaphores: HWDGE queues complete out of
        # order, so a shared counting semaphore waited at a prefix value is
        # unsafe. Each buffer's semaphore has one DMA in flight at a time
        # (reuse guarded by pe/act sems), so exact totals are race-free.
        slab_sems = [
            ctx.enter_context(nc.semaphore(f"slab_dma{j}")) for j in range(NSLAB)
        ]
        yout_sems = [
            ctx.enter_context(nc.semaphore(f"yout_dma{j}")) for j in range(NY)
        ]
        wband_sem = ctx.enter_context(nc.semaphore("wband_dma"))
        bias_sem = ctx.enter_context(nc.semaphore("bias_dma"))
        wm_sem = ctx.enter_context(nc.semaphore("wm_ready"))
        pe_sem = ctx.enter_context(nc.semaphore("pe_done"))
        act_sem = ctx.enter_context(nc.semaphore("act_done"))
        block = ctx.enter_context(nc.Block())

        @block.sync
        def _(sync):
            sync.dma_start(out=wband_sb.ap()[:], in_=wband_d.ap()[:]).then_inc(
                wband_sem, 16
            )
            sync.dma_start(out=bias_sb.ap()[:], in_=bias_d.ap()[:]).then_inc(
                bias_sem, 16
            )
            for i in range(TILES_PER_CORE):
                if i >= NSLAB:
                    # slab buffer reused: PE must be done reading tile i-NSLAB
                    sync.wait_ge(pe_sem, i - NSLAB + 1)
                sync.dma_start(
                    out=slabs[i % NSLAB].ap()[:],
                    in_=xslab_d.ap()[i].rearrange("h p d w -> p h d w"),
                ).then_inc(slab_sems[i % NSLAB], 16)

        @block.vector
        def _(vector):
            vector.memset(wmbuf.ap()[:], 0).then_inc(wm_sem, 1)

        @block.tensor
        def _(tensor):
            # Warmup: keep HAM busy during the initial DMA ramp. Zero
            # operands into psum bank 0, overwritten (start/stop) each time;
            # bank 0's first real use follows later in program order.
            tensor.wait_ge(wm_sem, 1)
            wm_lhs = wmbuf.ap()[:, 0:128]
            wm_rhs = wmbuf.ap()[:, 128:368].rearrange("p (a b) -> p a b", a=8)
            wm_out = psums[0].ap()[:, 0:8, :]
            for _w in range(N_WARMUP):
                tensor.matmul(wm_out, wm_lhs, wm_rhs, start=True, stop=True)

            tensor.wait_ge(wband_sem, 16)
            for g in range(n_groups):
                tiles = list(range(g * GROUP, min((g + 1) * GROUP, TILES_PER_CORE)))
                for i in tiles:
                    tensor.wait_ge(slab_sems[i % NSLAB], 16 * (i // NSLAB + 1))
                    if i >= NPSUM:
                        tensor.wait_ge(act_sem, i - NPSUM + 1)  # psum bank free
                r = 0
                for kd in range(K):
                    for kw in range(K):
                        for half in range(2):
                            last = r == N_ROUNDS - 1
                            for i in tiles:
                                mm = tensor.matmul(
                                    psums[i % NPSUM].ap()[:],
                                    wband_sb.ap()[:, r],
                                    slabs[i % NSLAB].ap()[
                                        :, half, kd : kd + OD, kw : kw + OW
                                    ],
                                    start=(r == 0),
                                    stop=last,
                                )
                                if last:
                                    mm.then_inc(pe_sem, 1)
                            r += 1

        @block.scalar
        def _(scalar):
            scalar.wait_ge(bias_sem, 16)  # bias loaded
            for i in range(TILES_PER_CORE):
                scalar.wait_ge(pe_sem, i + 1)
                if i >= NY:
                    scalar.wait_ge(yout_sems[i % NY], 16 * (i // NY))  # ysb free
                import concourse.mybir as mybir_

                scalar.activation(
                    ys[i % NY].ap()[:],
                    psums[i % NPSUM].ap()[:],
                    mybir_.ActivationFunctionType.Identity,
                    bias=bias_sb.ap()[:],
                ).then_inc(act_sem, 1)

        @block.gpsimd
        def _(gpsimd):
            for i in range(TILES_PER_CORE):
                gpsimd.wait_ge(act_sem, i + 1)
                gpsimd.dma_start(
                    out=yout_d.ap()[i], in_=ys[i % NY].ap()[:]
                ).then_inc(yout_sems[i % NY], 16)

    return nc


def _get_program():
    global _PROGRAM
    if _PROGRAM is None:
        _PROGRAM = _build_program()
    return _PROGRAM


def _prep_inputs(x, weight, bias):
    per_core_tiles = _tile_list()
    xslabs = []
    for core_tiles in per_core_tiles:
        xs = np.empty((TILES_PER_CORE, 2, 128, DD, WW), dtype=np.float32)
        for idx, (b, bt, bh) in enumerate(core_tiles):
            blk = x[b, :, 2 * bt : 2 * bt + 4, :, 2 * bh : 2 * bh + 4, :]
            # (c, t, d, h, w) -> (c, t, h, d, w)
            blk = blk.transpose(0, 1, 3, 2, 4)
            xs[idx] = blk.reshape(2, 8, 4, 4, DD, WW).reshape(2, 128, DD, WW)
        xslabs.append(xs)

    wband = np.zeros((N_ROUNDS, 8, 4, 4, 2, 2, CO), dtype=np.float32)
    r = 0
    for kd in range(K):
        for kw in range(K):
            for half in range(2):
                for dt in range(2):
                    for ti in range(4):
                        kt = ti - dt
                        if not (0 <= kt < K):
                            continue
                        for dh in range(2):
                            for hi in range(4):
                                kh = hi - dh
                                if not (0 <= kh < K):
                                    continue
                                # lhsT[(ci,ti,hi),(dt,dh,co)] = w[co, half*8+ci, kt, kd, kh, kw]
                                wband[r, :, ti, hi, dt, dh, :] = weight[
                                    :, half * 8 : half * 8 + 8, kt, kd, kh, kw
                                ].T
                r += 1
    # [rounds, k, m] -> partition-major [k, rounds, m]
    wbandT = np.ascontiguousarray(
        wband.reshape(N_ROUNDS, 128, 128).transpose(1, 0, 2)
    )

    bias128 = np.tile(bias.astype(np.float32), 4).reshape(128, 1)
    return xslabs, wbandT, bias128, per_core_tiles


def _scatter_outputs(youts, per_core_tiles):
    y = np.empty((B, CO, OT, OD, OH, OW), dtype=np.float32)
    for c in range(N_CORES):
        yc = youts[c].reshape(TILES_PER_CORE, 2, 2, CO, OD, OW)
        for idx, (b, bt, bh) in enumerate(per_core_tiles[c]):
            for dt in range(2):
                for dh in range(2):
                    y[b, :, 2 * bt + dt, :, 2 * bh + dh, :] = yc[idx, dt, dh]
    return y


def kernel(x, weight, bias):
    global LAST_EXEC_TIME_NS
    x = np.asarray(x, dtype=np.float32)
    weight = np.asarray(weight, dtype=np.float32)
    bias = np.asarray(bias, dtype=np.float32)

    xslabs, wbandT, bias128, per_core_tiles = _prep_inputs(x, weight, bias)
    nc = _get_program()

    from concourse.bass_utils import run_bass_kernel_spmd

    in_maps = [
        {"xslab": xslabs[c], "wband": wbandT, "bias128": bias128}
        for c in range(N_CORES)
    ]
    trace = os.environ.get("CONV4D_TRACE") == "1"
    res = run_bass_kernel_spmd(nc, in_maps, list(range(N_CORES)), trace=trace)
    LAST_EXEC_TIME_NS = res.exec_time_ns

    youts = [res.results[c]["yout"] for c in range(N_CORES)]
    return _scatter_outputs(youts, per_core_tiles)


# revision 15
# speedup vs baseline: 1.0647x; 1.0446x over previous
"""Conv4d (valid, stride 1) on 8 Trainium2 NeuronCores via Bass.

Problem (hardcoded):
  x      [2, 16, 16, 16, 32, 32]  (B, C_in, T, D, H, W)  fp32
  weight [32, 16, 3, 3, 3, 3]     (C_out, C_in, kT, kD, kH, kW)
  bias   [32]
  y      [2, 32, 14, 14, 30, 30]  (B, C_out, oT, oD, oH, oW)

Strategy: banded-matmul formulation on the PE array, fp32r (full-rate).
Each output tile is a 2x2 block in (oT, oH) x all of (oD, oW):
  matmul out[m, n], m = (dt2, dh2, c_out32) = 128, n = (od14, ow30) = 420
  K = 128 = (ci8, ti4, hi4): half of C_in x 4-wide t-window x 4-wide h-window.
  kt / kh taps live in a banded stationary weight (lhsT[k, m] = w[co, ci,
  ti-dt, kd, hi-dh, kw], zero off-band); kd, kw, c_in-half are 18
  PSUM-accumulation rounds whose input shifts are free-dim AP offsets.
  -> 72 useful MACs per PE column = 56.25% PE utilization.

Tiles: (b2, bt7, bh15) = 210, padded to 216 -> 27 per core (pads duplicate
real tiles; identical outputs make the scatter order-safe).

Host pre-packs per-core slabs so every DMA moves >=2KB contiguous per
partition; device pipeline: DMA-in -> 18 matmuls -> ACT bias-copy ->
DMA-out, with explicit semaphores (raw Bass, no Tile). Tiles are processed
in groups of 4 across 8 PSUM banks with rounds outer so consecutive
matmuls share the stationary operand; warmup matmuls keep the PE HAM
clock-gate open during the initial DMA ramp.
"""

import os

import numpy as np

B, C, TT, DD, HH, WW = 2, 16, 16, 16, 32, 32
CO = 32
K = 3
OT, OD, OH, OW = 14, 14, 30, 30
NBT, NBH = OT // 2, OH // 2  # 7, 15 output 2x2 blocks in (t, h)
N_CORES = 8
NTILES = B * NBT * NBH  # 210
TILES_PER_CORE = (NTILES + N_CORES - 1) // N_CORES  # 27
NPAD = TILES_PER_CORE * N_CORES  # 216
N_ROUNDS = K * K * 2  # (kd, kw, half) = 18

GROUP = 4  # tiles accumulated concurrently (one PSUM bank each)
NPSUM = 8
NSLAB = 2 * GROUP  # active group + prefetch group
NY = 4
N_WARMUP = 6  # PE warmup matmuls (N=240 fp32) during initial DMA ramp


def _group_sizes():
    # ramp-up groups so the first matmul only waits for slab 0, while later
    # groups of 4 amortize stationary-weight loads across PSUM banks
    sizes = [1, 2] + [GROUP] * ((TILES_PER_CORE - 3) // GROUP)
    assert sum(sizes) == TILES_PER_CORE, sizes
    return sizes

LAST_EXEC_TIME_NS = None
_PROGRAM = None


def _tile_list():
    tiles = [
        (b, bt, bh) for b in range(B) for bt in range(NBT) for bh in range(NBH)
    ]
    while len(tiles) < NPAD:
        tiles.append(tiles[0])
    return [tiles[c::N_CORES] for c in range(N_CORES)]


def _build_program():
    import concourse.bass as bass
    import concourse.mybir as mybir

    f32 = mybir.dt.float32
    f32r = mybir.dt.float32r

    nc = bass.Bass("TRN2", target_bir_lowering=False, debug=False)
    xslab_d = nc.dram_tensor(
        "xslab", [TILES_PER_CORE, 2, 128, DD, WW], f32r, kind="ExternalInput"
    )
    # partition-major: one DMA, 128 x 9216B contiguous lines
    wband_d = nc.dram_tensor(
        "wband", [128, N_ROUNDS, 128], f32r, kind="ExternalInput"
    )
    bias_d = nc.dram_tensor("bias128", [128, 1], f32, kind="ExternalInput")
    yout_d = nc.dram_tensor(
        "yout", [TILES_PER_CORE, 128, OD, OW], f32, kind="ExternalOutput"
    )

    n_groups = (TILES_PER_CORE + GROUP - 1) // GROUP

    from contextlib import ExitStack

    with ExitStack() as ctx:
        slabs = [
            ctx.enter_context(nc.sbuf_tensor(f"slab{j}", [128, 2, DD, WW], f32r))
            for j in range(NSLAB)
        ]
        wband_sb = ctx.enter_context(
            nc.sbuf_tensor("wband_sb", [128, N_ROUNDS, 128], f32r)
        )
        # warmup matmul operands (values irrelevant, zeroed for the sim)
        wmbuf = ctx.enter_context(nc.sbuf_tensor("wmbuf", [128, 368], f32))
        bias_sb = ctx.enter_context(nc.sbuf_tensor("bias_sb", [128, 1], f32))
        ys = [
            ctx.enter_context(nc.sbuf_tensor(f"ysb{j}", [128, OD, OW], f32))
            for j in range(NY)
        ]
        psums = [
            ctx.enter_context(nc.psum_tensor(f"ps{j}", [128, OD, OW], f32))
            for j in range(NPSUM)
        ]
        # Per-buffer DMA-completion semaphores: HWDGE queues complete out of
        # order, so a shared counting semaphore waited at a prefix value is
        # unsafe. Each buffer's semaphore has one DMA in flight at a time
        # (reuse guarded by pe/act sems), so exact totals are race-free.
        slab_sems = [
            ctx.enter_context(nc.semaphore(f"slab_dma{j}")) for j in range(NSLAB)
        ]
        yout_sems = [
            ctx.enter_context(nc.semaphore(f"yout_dma{j}")) for j in range(NY)
        ]
        wband_sem = ctx.enter_context(nc.semaphore("wband_dma"))
        bias_sem = ctx.enter_context(nc.semaphore("bias_dma"))
        wm_sem = ctx.enter_context(nc.semaphore("wm_ready"))
        pe_sem = ctx.enter_context(nc.semaphore("pe_done"))
        act_sem = ctx.enter_context(nc.semaphore("act_done"))
        block = ctx.enter_context(nc.Block())

        def _slab_dma(sync, i):
            if i >= NSLAB:
                # slab buffer reused: PE must be done reading tile i-NSLAB
                sync.wait_ge(pe_sem, i - NSLAB + 1)
            sync.dma_start(
                out=slabs[i % NSLAB].ap()[:],
                in_=xslab_d.ap()[i].rearrange("h p d w -> p h d w"),
            ).then_inc(slab_sems[i % NSLAB], 16)

        @block.sync
        def _(sync):
            _slab_dma(sync, 0)  # first compute dependency goes out first
            sync.dma_start(out=wband_sb.ap()[:], in_=wband_d.ap()[:]).then_inc(
                wband_sem, 16
            )
            sync.dma_start(out=bias_sb.ap()[:], in_=bias_d.ap()[:]).then_inc(
                bias_sem, 16
            )
            for i in range(1, TILES_PER_CORE):
                _slab_dma(sync, i)

        @block.vector
        def _(vector):
            vector.memset(wmbuf.ap()[:], 0).then_inc(wm_sem, 1)

        @block.tensor
        def _(tensor):
            # Warmup: keep HAM busy during the initial DMA ramp. Zero
            # operands into psum bank 0, overwritten (start/stop) each time;
            # bank 0's first real use follows later in program order.
            tensor.wait_ge(wm_sem, 1)
            wm_lhs = wmbuf.ap()[:, 0:128]
            wm_rhs = wmbuf.ap()[:, 128:368].rearrange("p (a b) -> p a b", a=8)
            wm_out = psums[0].ap()[:, 0:8, :]
            for _w in range(N_WARMUP):
                tensor.matmul(wm_out, wm_lhs, wm_rhs, start=True, stop=True)

            tensor.wait_ge(wband_sem, 16)
            base = 0
            for gsize in _group_sizes():
                tiles = list(range(base, base + gsize))
                base += gsize
                for i in tiles:
                    tensor.wait_ge(slab_sems[i % NSLAB], 16 * (i // NSLAB + 1))
                    if i >= NPSUM:
                        tensor.wait_ge(act_sem, i - NPSUM + 1)  # psum bank free
                r = 0
                for kd in range(K):
                    for kw in range(K):
                        for half in range(2):
                            last = r == N_ROUNDS - 1
                            for i in tiles:
                                mm = tensor.matmul(
                                    psums[i % NPSUM].ap()[:],
                                    wband_sb.ap()[:, r],
                                    slabs[i % NSLAB].ap()[
                                        :, half, kd : kd + OD, kw : kw + OW
                                    ],
                                    start=(r == 0),
                                    stop=last,
                                )
                                if last:
                                    mm.then_inc(pe_sem, 1)
                            r += 1

        @block.scalar
        def _(scalar):
            scalar.wait_ge(bias_sem, 16)  # bias loaded
            for i in range(TILES_PER_CORE):
                scalar.wait_ge(pe_sem, i + 1)
                if i >= NY:
                    scalar.wait_ge(yout_sems[i % NY], 16 * (i // NY))  # ysb free
                import concourse.mybir as mybir_

                scalar.activation(
                    ys[i % NY].ap()[:],
                    psums[i % NPSUM].ap()[:],
                    mybir_.ActivationFunctionType.Identity,
                    bias=bias_sb.ap()[:],
                ).then_inc(act_sem, 1)

        @block.gpsimd
        def _(gpsimd):
            for i in range(TILES_PER_CORE):
                gpsimd.wait_ge(act_sem, i + 1)
                gpsimd.dma_start(
                    out=yout_d.ap()[i], in_=ys[i % NY].ap()[:]
                ).then_inc(yout_sems[i % NY], 16)

    return nc


def _get_program():
    global _PROGRAM
    if _PROGRAM is None:
        _PROGRAM = _build_program()
    return _PROGRAM


def _prep_inputs(x, weight, bias):
    per_core_tiles = _tile_list()
    xslabs = []
    for core_tiles in per_core_tiles:
        xs = np.empty((TILES_PER_CORE, 2, 128, DD, WW), dtype=np.float32)
        for idx, (b, bt, bh) in enumerate(core_tiles):
            blk = x[b, :, 2 * bt : 2 * bt + 4, :, 2 * bh : 2 * bh + 4, :]
            # (c, t, d, h, w) -> (c, t, h, d, w)
            blk = blk.transpose(0, 1, 3, 2, 4)
            xs[idx] = blk.reshape(2, 8, 4, 4, DD, WW).reshape(2, 128, DD, WW)
        xslabs.append(xs)

    wband = np.zeros((N_ROUNDS, 8, 4, 4, 2, 2, CO), dtype=np.float32)
    r = 0
    for kd in range(K):
        for kw in range(K):
            for half in range(2):
                for dt in range(2):
                    for ti in range(4):
                        kt = ti - dt
                        if not (0 <= kt < K):
                            continue
                        for dh in range(2):
                            for hi in range(4):
                                kh = hi - dh
                                if not (0 <= kh < K):
                                    continue
                                # lhsT[(ci,ti,hi),(dt,dh,co)] = w[co, half*8+ci, kt, kd, kh, kw]
                                wband[r, :, ti, hi, dt, dh, :] = weight[
                                    :, half * 8 : half * 8 + 8, kt, kd, kh, kw
                                ].T
                r += 1
    # [rounds, k, m] -> partition-major [k, rounds, m]
    wbandT = np.ascontiguousarray(
        wband.reshape(N_ROUNDS, 128, 128).transpose(1, 0, 2)
    )

    bias128 = np.tile(bias.astype(np.float32), 4).reshape(128, 1)
    return xslabs, wbandT, bias128, per_core_tiles


def _scatter_outputs(youts, per_core_tiles):
    y = np.empty((B, CO, OT, OD, OH, OW), dtype=np.float32)
    for c in range(N_CORES):
        yc = youts[c].reshape(TILES_PER_CORE, 2, 2, CO, OD, OW)
        for idx, (b, bt, bh) in enumerate(per_core_tiles[c]):
            for dt in range(2):
                for dh in range(2):
                    y[b, :, 2 * bt + dt, :, 2 * bh + dh, :] = yc[idx, dt, dh]
    return y


def kernel(x, weight, bias):
    global LAST_EXEC_TIME_NS
    x = np.asarray(x, dtype=np.float32)
    weight = np.asarray(weight, dtype=np.float32)
    bias = np.asarray(bias, dtype=np.float32)

    xslabs, wbandT, bias128, per_core_tiles = _prep_inputs(x, weight, bias)
    nc = _get_program()

    from concourse.bass_utils import run_bass_kernel_spmd

    in_maps = [
        {"xslab": xslabs[c], "wband": wbandT, "bias128": bias128}
        for c in range(N_CORES)
    ]
    trace = os.environ.get("CONV4D_TRACE") == "1"
    res = run_bass_kernel_spmd(nc, in_maps, list(range(N_CORES)), trace=trace)
    LAST_EXEC_TIME_NS = res.exec_time_ns

    youts = [res.results[c]["yout"] for c in range(N_CORES)]
    return _scatter_outputs(youts, per_core_tiles)


# revision 16
# speedup vs baseline: 1.1001x; 1.0333x over previous
"""Conv4d (valid, stride 1) on 8 Trainium2 NeuronCores via Bass.

Problem (hardcoded):
  x      [2, 16, 16, 16, 32, 32]  (B, C_in, T, D, H, W)  fp32
  weight [32, 16, 3, 3, 3, 3]     (C_out, C_in, kT, kD, kH, kW)
  bias   [32]
  y      [2, 32, 14, 14, 30, 30]  (B, C_out, oT, oD, oH, oW)

Strategy: banded-matmul formulation on the PE array, fp32r (full-rate).
Each output tile is a 2x2 block in (oT, oH) x all of (oD, oW):
  matmul out[m, n], m = (dt2, dh2, c_out32) = 128, n = (od14, ow30) = 420
  K = 128 = (ci8, ti4, hi4): half of C_in x 4-wide t-window x 4-wide h-window.
  kt / kh taps live in a banded stationary weight (lhsT[k, m] = w[co, ci,
  ti-dt, kd, hi-dh, kw], zero off-band); (half, kd, kw) are 18
  PSUM-accumulation rounds whose input shifts are free-dim AP offsets.
  -> 72 useful MACs per PE column = 56.25% PE utilization.

Tiles: (b2, bt7, bh15) = 210, padded to 216 -> 27 per core (pads duplicate
real tiles; identical outputs make the scatter order-safe).

Host pre-packs per-core slabs so every DMA moves 2KB+ contiguous per
partition; device pipeline: DMA-in -> 18 matmuls -> ACT bias-copy ->
DMA-out with explicit semaphores (raw Bass, no Tile). Warmup matmuls hold
the PE HAM clock-gate open through the initial DMA ramp; tile 0's slab is
DMA'd in halves (c_in halves) with dedicated semaphores so its half-0
rounds start as early as possible.
"""

import os

import numpy as np

B, C, TT, DD, HH, WW = 2, 16, 16, 16, 32, 32
CO = 32
K = 3
OT, OD, OH, OW = 14, 14, 30, 30
NBT, NBH = OT // 2, OH // 2  # 7, 15 output 2x2 blocks in (t, h)
N_CORES = 8
NTILES = B * NBT * NBH  # 210
TILES_PER_CORE = (NTILES + N_CORES - 1) // N_CORES  # 27
NPAD = TILES_PER_CORE * N_CORES  # 216
N_ROUNDS = K * K * 2  # (half, kd, kw) = 18

NSLAB = 3
NPSUM = 4
NY = 3
N_WARMUP = 10  # PE warmup matmuls (N=240 fp32, ~400ns each) during DMA ramp

LAST_EXEC_TIME_NS = None
_PROGRAM = None


def _tile_list():
    tiles = [
        (b, bt, bh) for b in range(B) for bt in range(NBT) for bh in range(NBH)
    ]
    while len(tiles) < NPAD:
        tiles.append(tiles[0])
    return [tiles[c::N_CORES] for c in range(N_CORES)]


def _build_program():
    import concourse.bass as bass
    import concourse.mybir as mybir

    f32 = mybir.dt.float32
    f32r = mybir.dt.float32r

    nc = bass.Bass("TRN2", target_bir_lowering=False, debug=False)
    xslab_d = nc.dram_tensor(
        "xslab", [TILES_PER_CORE, 2, 128, DD, WW], f32r, kind="ExternalInput"
    )
    # partition-major: one DMA, 128 x 9216B contiguous lines
    wband_d = nc.dram_tensor(
        "wband", [128, N_ROUNDS, 128], f32r, kind="ExternalInput"
    )
    bias_d = nc.dram_tensor("bias128", [128, 1], f32, kind="ExternalInput")
    yout_d = nc.dram_tensor(
        "yout", [TILES_PER_CORE, 128, OD, OW], f32, kind="ExternalOutput"
    )

    from contextlib import ExitStack

    with ExitStack() as ctx:
        slabs = [
            ctx.enter_context(nc.sbuf_tensor(f"slab{j}", [128, 2, DD, WW], f32r))
            for j in range(NSLAB)
        ]
        wband_sb = ctx.enter_context(
            nc.sbuf_tensor("wband_sb", [128, N_ROUNDS, 128], f32r)
        )
        # warmup matmul operands (values irrelevant, zeroed for the sim)
        wmbuf = ctx.enter_context(nc.sbuf_tensor("wmbuf", [128, 368], f32))
        bias_sb = ctx.enter_context(nc.sbuf_tensor("bias_sb", [128, 1], f32))
        ys = [
            ctx.enter_context(nc.sbuf_tensor(f"ysb{j}", [128, OD, OW], f32))
            for j in range(NY)
        ]
        psums = [
            ctx.enter_context(nc.psum_tensor(f"ps{j}", [128, OD, OW], f32))
            for j in range(NPSUM)
        ]
        # Per-buffer DMA-completion semaphores: HWDGE queues complete out of
        # order, so a shared counting semaphore waited at a prefix value is
        # unsafe. Waits here always target the exact total of all DMAs ever
        # issued on that semaphore, which is order-independent.
        slab_sems = [
            ctx.enter_context(nc.semaphore(f"slab_dma{j}")) for j in range(NSLAB)
        ]
        s0h_sems = [
            ctx.enter_context(nc.semaphore(f"slab0_h{h}")) for h in range(2)
        ]
        yout_sems = [
            ctx.enter_context(nc.semaphore(f"yout_dma{j}")) for j in range(NY)
        ]
        wband_sem = ctx.enter_context(nc.semaphore("wband_dma"))
        bias_sem = ctx.enter_context(nc.semaphore("bias_dma"))
        wm_sem = ctx.enter_context(nc.semaphore("wm_ready"))
        pe_sem = ctx.enter_context(nc.semaphore("pe_done"))
        act_sem = ctx.enter_context(nc.semaphore("act_done"))
        block = ctx.enter_context(nc.Block())

        # count of DMA increments per slab buffer, for exact-total waits
        slab_incs = [0] * NSLAB

        @block.sync
        def _(sync):
            # tile 0 half 0 -> wband -> tile 0 half 1 -> bias -> rest
            sync.dma_start(
                out=slabs[0].ap()[:, 0], in_=xslab_d.ap()[0, 0]
            ).then_inc(s0h_sems[0], 16)
            sync.dma_start(out=wband_sb.ap()[:], in_=wband_d.ap()[:]).then_inc(
                wband_sem, 16
            )
            sync.dma_start(
                out=slabs[0].ap()[:, 1], in_=xslab_d.ap()[0, 1]
            ).then_inc(s0h_sems[1], 16)
            sync.dma_start(out=bias_sb.ap()[:], in_=bias_d.ap()[:]).then_inc(
                bias_sem, 16
            )
            for i in range(1, TILES_PER_CORE):
                if i >= NSLAB:
                    # slab buffer reused: PE must be done reading tile i-NSLAB
                    sync.wait_ge(pe_sem, i - NSLAB + 1)
                sync.dma_start(
                    out=slabs[i % NSLAB].ap()[:],
                    in_=xslab_d.ap()[i].rearrange("h p d w -> p h d w"),
                ).then_inc(slab_sems[i % NSLAB], 16)
                slab_incs[i % NSLAB] += 16

        # snapshot running totals per tile for the PE waits
        slab_wait_vals = {}
        incs = [0] * NSLAB
        for i in range(1, TILES_PER_CORE):
            incs[i % NSLAB] += 16
            slab_wait_vals[i] = incs[i % NSLAB]

        @block.tensor
        def _(tensor):
            # Warmup: keep HAM busy during the initial DMA ramp. Zero
            # operands into psum bank 0, overwritten (start/stop) each time;
            # bank 0's first real use follows later in program order.
            tensor.wait_ge(wm_sem, 1)
            wm_lhs = wmbuf.ap()[:, 0:128]
            wm_rhs = wmbuf.ap()[:, 128:368].rearrange("p (a b) -> p a b", a=8)
            wm_out = psums[0].ap()[:, 0:8, :]
            for _w in range(N_WARMUP):
                tensor.matmul(wm_out, wm_lhs, wm_rhs, start=True, stop=True)

            tensor.wait_ge(wband_sem, 16)
            for i in range(TILES_PER_CORE):
                if i == 0:
                    tensor.wait_ge(s0h_sems[0], 16)
                else:
                    tensor.wait_ge(slab_sems[i % NSLAB], slab_wait_vals[i])
                if i >= NPSUM:
                    tensor.wait_ge(act_sem, i - NPSUM + 1)  # psum bank free
                ps = psums[i % NPSUM].ap()[:]
                sl = slabs[i % NSLAB].ap()
                r = 0
                for half in range(2):
                    if i == 0 and half == 1:
                        tensor.wait_ge(s0h_sems[1], 16)
                    for kd in range(K):
                        for kw in range(K):
                            last = r == N_ROUNDS - 1
                            mm = tensor.matmul(
                                ps,
                                wband_sb.ap()[:, r],
                                sl[:, half, kd : kd + OD, kw : kw + OW],
                                start=(r == 0),
                                stop=last,
                            )
                            if last:
                                mm.then_inc(pe_sem, 1)
                            r += 1

        @block.scalar
        def _(scalar):
            import concourse.mybir as mybir_

            scalar.wait_ge(bias_sem, 16)  # bias loaded
            for i in range(TILES_PER_CORE):
                scalar.wait_ge(pe_sem, i + 1)
                if i >= NY:
                    scalar.wait_ge(yout_sems[i % NY], 16 * (i // NY))  # ysb free
                scalar.activation(
                    ys[i % NY].ap()[:],
                    psums[i % NPSUM].ap()[:],
                    mybir_.ActivationFunctionType.Identity,
                    bias=bias_sb.ap()[:],
                ).then_inc(act_sem, 1)

        @block.vector
        def _(vector):
            vector.memset(wmbuf.ap()[:], 0).then_inc(wm_sem, 1)

        @block.gpsimd
        def _(gpsimd):
            for i in range(TILES_PER_CORE):
                gpsimd.wait_ge(act_sem, i + 1)
                gpsimd.dma_start(
                    out=yout_d.ap()[i], in_=ys[i % NY].ap()[:]
                ).then_inc(yout_sems[i % NY], 16)

    return nc


def _get_program():
    global _PROGRAM
    if _PROGRAM is None:
        _PROGRAM = _build_program()
    return _PROGRAM


def _prep_inputs(x, weight, bias):
    per_core_tiles = _tile_list()
    xslabs = []
    for core_tiles in per_core_tiles:
        xs = np.empty((TILES_PER_CORE, 2, 128, DD, WW), dtype=np.float32)
        for idx, (b, bt, bh) in enumerate(core_tiles):
            blk = x[b, :, 2 * bt : 2 * bt + 4, :, 2 * bh : 2 * bh + 4, :]
            # (c, t, d, h, w) -> (c, t, h, d, w)
            blk = blk.transpose(0, 1, 3, 2, 4)
            xs[idx] = blk.reshape(2, 8, 4, 4, DD, WW).reshape(2, 128, DD, WW)
        xslabs.append(xs)

    wband = np.zeros((N_ROUNDS, 8, 4, 4, 2, 2, CO), dtype=np.float32)
    r = 0
    for half in range(2):
        for kd in range(K):
            for kw in range(K):
                for dt in range(2):
                    for ti in range(4):
                        kt = ti - dt
                        if not (0 <= kt < K):
                            continue
                        for dh in range(2):
                            for hi in range(4):
                                kh = hi - dh
                                if not (0 <= kh < K):
                                    continue
                                # lhsT[(ci,ti,hi),(dt,dh,co)] = w[co, half*8+ci, kt, kd, kh, kw]
                                wband[r, :, ti, hi, dt, dh, :] = weight[
                                    :, half * 8 : half * 8 + 8, kt, kd, kh, kw
                                ].T
                r += 1
    # [rounds, k, m] -> partition-major [k, rounds, m]
    wbandT = np.ascontiguousarray(
        wband.reshape(N_ROUNDS, 128, 128).transpose(1, 0, 2)
    )

    bias128 = np.tile(bias.astype(np.float32), 4).reshape(128, 1)
    return xslabs, wbandT, bias128, per_core_tiles


def _scatter_outputs(youts, per_core_tiles):
    y = np.empty((B, CO, OT, OD, OH, OW), dtype=np.float32)
    for c in range(N_CORES):
        yc = youts[c].reshape(TILES_PER_CORE, 2, 2, CO, OD, OW)
        for idx, (b, bt, bh) in enumerate(per_core_tiles[c]):
            for dt in range(2):
                for dh in range(2):
                    y[b, :, 2 * bt + dt, :, 2 * bh + dh, :] = yc[idx, dt, dh]
    return y


def kernel(x, weight, bias):
    global LAST_EXEC_TIME_NS
    x = np.asarray(x, dtype=np.float32)
    weight = np.asarray(weight, dtype=np.float32)
    bias = np.asarray(bias, dtype=np.float32)

    xslabs, wbandT, bias128, per_core_tiles = _prep_inputs(x, weight, bias)
    nc = _get_program()

    from concourse.bass_utils import run_bass_kernel_spmd

    in_maps = [
        {"xslab": xslabs[c], "wband": wbandT, "bias128": bias128}
        for c in range(N_CORES)
    ]
    trace = os.environ.get("CONV4D_TRACE") == "1"
    res = run_bass_kernel_spmd(nc, in_maps, list(range(N_CORES)), trace=trace)
    LAST_EXEC_TIME_NS = res.exec_time_ns

    youts = [res.results[c]["yout"] for c in range(N_CORES)]
    return _scatter_outputs(youts, per_core_tiles)
